# revision 30
# baseline (speedup 1.0000x reference)
"""Trainium2 Bass kernel for the 2-layer cross-attention module.

Sharding: data-parallel over batch B=16 -> 2 batch elements per core, 8 cores,
no collectives. Per core the algebra is restructured by linearity so the two
giant [T,NZ]@[NZ,NZ] projections disappear:

  scores[t,h] = sum_i keys[t,i] * qk[i,h],   qk[i,h] = sum_{d in head h} Wk[i,hd] q[hd]
  vals[hd]    = sum_i wvals[h,i] * Wv[i,hd], wvals[h,i] = sum_t p[t,h] values[t,i]

The small linear chain runs in "row form": activations [2, 1024] with the
column-layout x tiles as the (cheap) stationary operand and the weight matrix
moving in N=512 chunks — 8 LDWEIGHTS + 16 dense matmuls per linear instead of
64+64. The final layer's pred-MLP/LayerNorms are dead code in the reference
(query is discarded) and are skipped. All matmul inputs bf16 (f32 PSUM);
softmax/layernorm in f32.
"""

import numpy as np
import ml_dtypes

import concourse.mybir as mybir
from concourse import bacc, tile
from concourse.bass_utils import run_bass_kernel_spmd
from concourse.masks import make_identity

F32 = mybir.dt.float32
BF16 = mybir.dt.bfloat16
ALU = mybir.AluOpType
AXX = mybir.AxisListType
AF = mybir.ActivationFunctionType

B, T, NZ, H, DK = 16, 2048, 1024, 8, 128
NCORES, BP = 8, 2
NT, NI = T // 128, NZ // 128  # 16, 8
SCALE = float(1.0 / np.sqrt(DK))  # * attention_temperature (1.0)
EPS = 1e-5
N_LAYERS = 2
LRELU = 0.2

LAST_RESULT = {}  # test harness introspection: exec_time_ns etc.


def _np(x):
    return np.asarray(x)


def _bf(x):
    return np.ascontiguousarray(np.asarray(x, dtype=np.float32).astype(ml_dtypes.bfloat16))


def _f32(x):
    return np.ascontiguousarray(np.asarray(x), dtype=np.float32)


def _host_prep(values, keys, query_input, start_ind, end_ind, params):
    """Marshal full inputs into per-core shard dicts + build config."""
    values = _np(values)
    keys = _np(keys)
    query_input = _np(query_input)
    start_ind = _f32(start_ind)
    end_ind = _f32(end_ind)

    lin_w = {}
    lin_b = {}

    def add_lin(name, p):
        W, b = p
        lin_w[name] = _bf(_f32(W))
        lin_b[name] = _f32(b).reshape(1, -1)

    for i, p in enumerate(params["query_net"]):
        add_lin(f"qn{i}", p)
    for l, lp in enumerate(params["layers"]):
        add_lin(f"wq{l}", lp["q"])
        add_lin(f"wv{l}", lp["v"])
        add_lin(f"wao{l}", lp["attn_out"])
        if l != N_LAYERS - 1:  # last layer's pred MLP is dead code
            for i, p in enumerate(lp["pred"]):
                add_lin(f"pred{l}_{i}", p)
        Wk, bk = lp["k"]
        lin_w[f"wkT{l}"] = _bf(_f32(Wk).T)
        lin_b[f"bk{l}"] = _f32(bk).reshape(1, -1)
    add_lin("wout", params["out"])

    bias_nonzero = {k: bool(np.any(v)) for k, v in lin_b.items()}

    tt = np.arange(T, dtype=np.float32)
    mask = (tt[None, :] < np.floor(start_ind)[:, None]) | (
        tt[None, :] > np.ceil(end_ind)[:, None]
    )
    fully = mask.all(axis=1)
    maskadd = np.where(mask, np.float32(-1e30), np.float32(0.0)).astype(np.float32)
    maskadd[fully] = 0.0  # fully-masked row -> uniform softmax (matches reference)
    mask_nontrivial = bool(maskadd.any())

    cfg = {"bias_nonzero": bias_nonzero, "mask": mask_nontrivial}

    in_maps = []
    for c in range(NCORES):
        sl_ = slice(BP * c, BP * (c + 1))
        m = {
            "keysT": _bf(keys[sl_].transpose(0, 2, 1)),          # [BP, NZ, T]
            "values": _bf(values[sl_]),                          # [BP, T, NZ]
            "qin": _bf(query_input[sl_].T.reshape(16, 128, BP).transpose(1, 0, 2)),
        }
        for k, v in lin_w.items():
            m[k] = v
        for k, v in lin_b.items():
            if bias_nonzero[k]:
                if k.startswith("bk"):
                    m[k + "_col"] = v.reshape(H, DK).T.copy()
                else:
                    m[k + "_bias"] = v
        if mask_nontrivial:
            m["maskadd"] = np.repeat(maskadd[sl_][:, None, :], H, axis=1).copy()
        in_maps.append(m)
    return in_maps, cfg


def build(cfg):
    bias_nz = cfg["bias_nonzero"]
    nc = bacc.Bacc("TRN2", target_bir_lowering=False, debug=False)

    keysT_d = nc.dram_tensor("keysT", [BP, NZ, T], BF16, kind="ExternalInput")
    values_d = nc.dram_tensor("values", [BP, T, NZ], BF16, kind="ExternalInput")
    qin_d = nc.dram_tensor("qin", [128, 16, BP], BF16, kind="ExternalInput")

    w_d = {}
    b_d = {}

    def decl_lin(name, din):
        w_d[name] = nc.dram_tensor(name, [din, NZ], BF16, kind="ExternalInput")
        if bias_nz.get(name, False):
            b_d[name] = nc.dram_tensor(name + "_bias", [1, NZ], F32, kind="ExternalInput")

    decl_lin("qn0", 2 * NZ)
    for i in range(1, 5):
        decl_lin(f"qn{i}", NZ)
    for l in range(N_LAYERS):
        decl_lin(f"wq{l}", NZ)
        decl_lin(f"wv{l}", NZ)
        decl_lin(f"wao{l}", NZ)
        if l != N_LAYERS - 1:
            for i in range(4):
                decl_lin(f"pred{l}_{i}", NZ)
        w_d[f"wkT{l}"] = nc.dram_tensor(f"wkT{l}", [NZ, NZ], BF16, kind="ExternalInput")
        if bias_nz.get(f"bk{l}", False):
            b_d[f"bk{l}"] = nc.dram_tensor(f"bk{l}_col", [128, H], F32, kind="ExternalInput")
    decl_lin("wout", NZ)
    if cfg["mask"]:
        mask_d = nc.dram_tensor("maskadd", [BP, H, T], F32, kind="ExternalInput")

    out_d = nc.dram_tensor("out", [BP, NZ], F32, kind="ExternalOutput")
    attw_d = nc.dram_tensor("attw", [BP, T], F32, kind="ExternalOutput")

    from contextlib import ExitStack

    with tile.TileContext(nc) as tc, ExitStack() as ctx:
        singles = ctx.enter_context(tc.tile_pool(name="singles", bufs=1))
        wpool = ctx.enter_context(tc.tile_pool(name="wpool", bufs=9))
        apool = ctx.enter_context(tc.tile_pool(name="apool", bufs=36))
        rpool = ctx.enter_context(tc.tile_pool(name="rpool", bufs=4))
        ppool = ctx.enter_context(tc.tile_pool(name="ppool", bufs=2))
        mpool = ctx.enter_context(tc.tile_pool(name="mpool", bufs=3))
        ps_wide = ctx.enter_context(tc.tile_pool(name="ps_wide", bufs=4, space="PSUM"))
        ps_mm2 = ctx.enter_context(tc.tile_pool(name="ps_mm2", bufs=1, space="PSUM"))
        ps_tp = ctx.enter_context(tc.tile_pool(name="ps_tp", bufs=3, space="PSUM"))

        # ---- constants -------------------------------------------------
        id_b = singles.tile([128, 128], BF16, tag="id_b")
        make_identity(nc, id_b)
        ones2 = singles.tile([1, 2], F32, tag="ones2")
        nc.vector.memset(ones2, 1.0)
        ones16 = singles.tile([1, 16], F32, tag="ones16")
        nc.vector.memset(ones16, 1.0)
        sel8 = singles.tile([8, 1], BF16, tag="sel8")
        nc.vector.memset(sel8, 1.0 / H)
        eps_t = singles.tile([BP, 1], F32, tag="eps")
        nc.vector.memset(eps_t, EPS)

        # ---- persistent inputs (DMA deferred; weights queue first) -----
        keysT_sb3 = [singles.tile([128, NI, T], BF16, tag=f"kT{b}", name=f"kT{b}")
                     for b in range(BP)]
        keysT_sb = [[keysT_sb3[b][:, it, :] for it in range(NI)] for b in range(BP)]

        def load_keysT(b):
            nc.sync.dma_start(out=keysT_sb3[b],
                              in_=keysT_d[b].rearrange("(it p) t -> p it t", p=128))

        vspool = ctx.enter_context(tc.tile_pool(name="vspool", bufs=3))

        def stream_values(b, c, half):
            """[128, 8, 512] bf16: tt in [8*half, 8*half+8), cols c*512.."""
            v = vspool.tile([128, 8, 512], BF16, tag="vs", name="vs")
            nc.sync.dma_start(
                out=v,
                in_=values_d[b, half * 1024:(half + 1) * 1024,
                             c * 512:(c + 1) * 512].rearrange(
                                 "(tt p) i -> p tt i", p=128))
            return v

        qin_sb = singles.tile([128, 16, BP], BF16, tag="qin")
        nc.sync.dma_start(out=qin_sb, in_=qin_d[:, :, :])
        if cfg["mask"]:
            mask_sb = [singles.tile([H, T], F32, tag=f"mask{b}", name=f"mask{b}")
                       for b in range(BP)]
            for b in range(BP):
                nc.sync.dma_start(out=mask_sb[b], in_=mask_d[b, :, :])

        # ---- helpers ---------------------------------------------------
        def load_w(name, nk):
            segs = []
            for s0 in range(0, nk, 4):
                w = wpool.tile([128, 4, NZ], BF16, tag="w", name=f"w_{name}_{s0}")
                nc.sync.dma_start(
                    out=w,
                    in_=w_d[name][s0 * 128:(s0 + 4) * 128, :].rearrange(
                        "(k p) n -> p k n", p=128))
                segs.append(w)
            return lambda kt: segs[kt // 4][:, kt % 4, :]

        def load_brow(name):
            if not bias_nz.get(name, False):
                return None
            t = mpool.tile([1, NZ], F32, tag="brow", bufs=3, name=f"b_{name}")
            nc.sync.dma_start(out=t, in_=b_d[name][:, :])
            return t

        def linear_row(x_cols, name, nk, act=None, want_bf=True, want32=False):
            """Row-form linear: x_cols (nk [128,2] bf16 APs) @ W -> rows [2, NZ].

            Returns (row_bf16 | None, row_f32 | None).
            """
            wts = load_w(name, nk)
            brow = load_brow(name)
            ch = [ps_wide.tile([BP, 512], F32, tag="wide", name=f"r{c}_{name}")
                  for c in range(2)]
            for kt in range(nk):
                w = wts(kt)
                for c in range(2):
                    nc.tensor.matmul(ch[c], lhsT=x_cols[kt],
                                     rhs=w[:, c * 512:(c + 1) * 512],
                                     start=(kt == 0),
                                     stop=(kt == nk - 1 and brow is None))
            if brow is not None:
                for c in range(2):
                    nc.tensor.matmul(ch[c], lhsT=ones2,
                                     rhs=brow[:, c * 512:(c + 1) * 512],
                                     start=False, stop=True)
            # evict: ACT psum->SBUF with cast; lrelu (if any) is applied later
            # on the transposed col tiles to keep this boundary short
            r32 = None
            if want32:
                r32 = rpool.tile([BP, NZ], F32, tag="r32", bufs=4, name=f"r32_{name}")
                for c in range(2):
                    nc.scalar.copy(out=r32[:, c * 512:(c + 1) * 512], in_=ch[c])
            rbf = None
            if want_bf:
                rbf = rpool.tile([BP, NZ], BF16, tag="rbf", bufs=3, name=f"rbf_{name}")
                if r32 is not None:
                    nc.vector.tensor_copy(out=rbf, in_=r32)
                else:
                    for c in range(2):
                        nc.scalar.copy(out=rbf[:, c * 512:(c + 1) * 512], in_=ch[c])
            return rbf, r32

        def to_cols(row_bf, n=NI, act=None):
            """[2, n*128] bf16 row -> list of n [128, 2] bf16 col tiles."""
            cols = []
            for kt in range(n):
                tp = ps_tp.tile([128, BP], BF16, tag="tp", name="tp_c")
                nc.tensor.transpose(out=tp, in_=row_bf[:, kt * 128:(kt + 1) * 128],
                                    identity=id_b[0:BP, 0:BP])
                t = apool.tile([128, BP], BF16, tag="act", name="col")
                nc.vector.tensor_copy(out=t, in_=tp)
                if act == "lrelu":
                    nc.vector.scalar_tensor_tensor(out=t, in0=t, scalar=LRELU,
                                                   in1=t, op0=ALU.mult, op1=ALU.max)
                cols.append(t)
            return cols

        def ln_row(r32, act_bf=True, want32=False):
            """LayerNorm over the free dim of [2, NZ] f32 rows."""
            stats = mpool.tile([BP, 2, 6], F32, tag="lnst", name="lnst")
            for c in range(2):
                nc.vector.bn_stats(out=stats[:, c, :], in_=r32[:, c * 512:(c + 1) * 512])
            mv = mpool.tile([BP, 2], F32, tag="lnmv", name="lnmv")
            nc.vector.bn_aggr(out=mv, in_=stats)
            sd = mpool.tile([BP, 1], F32, tag="lnsd", name="lnsd")
            nc.scalar.activation(out=sd, in_=mv[:, 1:2], func=AF.Sqrt,
                                 bias=eps_t, scale=1.0)
            rinv = mpool.tile([BP, 1], F32, tag="lnri", name="lnri")
            nc.vector.reciprocal(rinv, sd)
            yb = y32 = None
            if act_bf:
                yb = rpool.tile([BP, NZ], BF16, tag="rbf", bufs=3, name="ln_rbf")
                nc.vector.tensor_scalar(out=yb, in0=r32, scalar1=mv[:, 0:1],
                                        scalar2=rinv, op0=ALU.subtract, op1=ALU.mult)
            if want32:
                y32 = rpool.tile([BP, NZ], F32, tag="r32", bufs=4, name="ln_r32")
                nc.vector.tensor_scalar(out=y32, in0=r32, scalar1=mv[:, 0:1],
                                        scalar2=rinv, op0=ALU.subtract, op1=ALU.mult)
            return yb, y32

        # ---- query_net -------------------------------------------------
        x_cols = [qin_sb[:, kt, :] for kt in range(16)]
        for i in range(4):
            rbf, _ = linear_row(x_cols, f"qn{i}", 16 if i == 0 else NI)
            x_cols = to_cols(rbf, act="lrelu")
        query_bf, query_32 = linear_row(x_cols, "qn4", NI, want_bf=True, want32=True)

        raw_bf = None
        pnorm = [None, None]

        for l in range(N_LAYERS):
            last = l == N_LAYERS - 1
            # q projection
            q_cols = to_cols(query_bf)
            q_rbf, _ = linear_row(q_cols, f"wq{l}", NI)
            q_cc = to_cols(q_rbf)

            # qk^T = blockdiag(q) @ WkT   [8, 1024] per batch
            wkT = load_w(f"wkT{l}", NI)
            if l == 0:
                load_keysT(0)
                load_keysT(1)
            qk_col = [[None] * NI for _ in range(BP)]
            for b in range(BP):
                qblk = []
                for kt in range(H):
                    qb = apool.tile([128, H], BF16, tag="qblk", bufs=20, name="qb")
                    nc.vector.memset(qb, 0.0)
                    nc.vector.tensor_copy(out=qb[:, kt:kt + 1],
                                          in_=q_cc[kt][:, b:b + 1])
                    qblk.append(qb)
                qkT_sb = mpool.tile([H, NZ], BF16, tag="qkT", bufs=2, name="qkT")
                for c in range(2):
                    ps = ps_wide.tile([H, 512], F32, tag="wide", name="ps_qkT")
                    for kt in range(H):
                        nc.tensor.matmul(ps, lhsT=qblk[kt],
                                         rhs=wkT(kt)[:, c * 512:(c + 1) * 512],
                                         start=(kt == 0), stop=(kt == H - 1))
                    nc.vector.tensor_copy(out=qkT_sb[:, c * 512:(c + 1) * 512], in_=ps)
                for it in range(NI):
                    tp = ps_tp.tile([128, H], BF16, tag="tp", name="tp_qk")
                    nc.tensor.transpose(out=tp, in_=qkT_sb[:, it * 128:(it + 1) * 128],
                                        identity=id_b[0:H, 0:H])
                    qc = apool.tile([128, H], BF16, tag="qkcol", bufs=20, name="qkc")
                    nc.vector.tensor_copy(out=qc, in_=tp)
                    qk_col[b][it] = qc

            # scb: per-head constant from k-bias (zero in practice -> skipped)
            scb_sb = [None, None]
            if bias_nz.get(f"bk{l}", False):
                bkcol = mpool.tile([128, H], F32, tag="bkcol", name="bkcol")
                nc.sync.dma_start(out=bkcol, in_=b_d[f"bk{l}"][:, :])
                ones512 = singles.tile([1, 512], F32, tag="ones512")
                nc.vector.memset(ones512, 1.0)
                bk_bf = mpool.tile([128, H], BF16, tag="bkbf", name="bkbf")
                nc.vector.tensor_copy(out=bk_bf, in_=bkcol)
                for b in range(BP):
                    ps = ps_mm2.tile([1, H], F32, tag="mm2", name="ps_scb")
                    for h in range(H):
                        nc.tensor.matmul(ps[:, h:h + 1],
                                         lhsT=q_cc[h][:, b:b + 1],
                                         rhs=bk_bf[:, h:h + 1],
                                         start=(h == 0), stop=(h == H - 1))
                    s_ = mpool.tile([1, H], F32, tag="scb", name="scb")
                    nc.vector.tensor_copy(out=s_, in_=ps)
                    scb_sb[b] = s_

            # scores -> softmax -> pT -> wvals, pipelined per batch.
            # Softmax without max-subtraction: scores are O(1) here (and a
            # -1e30 mask exps to exactly 0), so exp cannot overflow; exp is
            # fused into the per-chunk PSUM eviction with accumulated partial
            # sums, and the 1/sum normalization is folded into the wvals
            # eviction and the attw selector.
            wv_sb = []
            for b in range(BP):
                probs = ppool.tile([H, T], BF16, tag="probs", name="probs")
                psums = mpool.tile([H, 4], F32, tag="psums", name="psums")
                for c in range(4):
                    ps = ps_wide.tile([H, 512], F32, tag="wide", name="ps_sc")
                    for it in range(NI):
                        nc.tensor.matmul(ps,
                                         lhsT=qk_col[b][it],
                                         rhs=keysT_sb[b][it][:, c * 512:(c + 1) * 512],
                                         start=(it == 0),
                                         stop=(it == NI - 1 and scb_sb[b] is None))
                    if scb_sb[b] is not None:
                        nc.tensor.matmul(ps, lhsT=scb_sb[b], rhs=ones512,
                                         start=False, stop=True)
                    if cfg["mask"]:
                        nc.vector.tensor_add(ps, ps,
                                             mask_sb[b][:, c * 512:(c + 1) * 512])
                    nc.scalar.activation(out=probs[:, c * 512:(c + 1) * 512], in_=ps,
                                         func=AF.Exp, bias=0.0, scale=SCALE,
                                         accum_out=psums[:, c:c + 1])
                ssum = mpool.tile([H, 1], F32, tag="ssum", name="ssum")
                nc.vector.reduce_sum(ssum, psums, axis=AXX.X)
                rec = mpool.tile([H, 1], F32, tag="rec", bufs=4, name="rec")
                nc.vector.reciprocal(rec, ssum)
                pnorm[b] = probs

                if last:
                    selr = mpool.tile([H, 1], BF16, tag="selr", name="selr")
                    nc.vector.tensor_scalar_mul(selr, rec, 1.0 / H)
                    for c in range(4):
                        ps = ps_wide.tile([1, 512], F32, tag="wide", name="ps_aw")
                        nc.tensor.matmul(ps, lhsT=selr,
                                         rhs=pnorm[b][:, c * 512:(c + 1) * 512],
                                         start=True, stop=True)
                        awc = mpool.tile([1, 512], F32, tag="awc", bufs=1, name="awc")
                        nc.vector.tensor_copy(out=awc, in_=ps)
                        nc.sync.dma_start(out=attw_d[b:b + 1, c * 512:(c + 1) * 512],
                                          in_=awc)

                pT_b = []
                for tt in range(NT):
                    tp = ps_tp.tile([128, H], BF16, tag="tp", name="tp_p")
                    nc.tensor.transpose(out=tp,
                                        in_=pnorm[b][:, tt * 128:(tt + 1) * 128],
                                        identity=id_b[0:H, 0:H])
                    t = mpool.tile([128, H], BF16, tag="pT", bufs=36, name="pT_t")
                    nc.vector.tensor_copy(out=t, in_=tp)
                    pT_b.append(t)
                wv_b = mpool.tile([H, NZ], BF16, tag="wv", bufs=2, name="wv_b")
                for c in range(2):
                    ps = ps_wide.tile([H, 512], F32, tag="wide", name="ps_wv")
                    for half in range(2):
                        vt = stream_values(b, c, half)
                        for t8 in range(8):
                            tt = half * 8 + t8
                            nc.tensor.matmul(ps,
                                             lhsT=pT_b[tt],
                                             rhs=vt[:, t8, :],
                                             start=(tt == 0), stop=(tt == NT - 1))
                    nc.vector.tensor_scalar(out=wv_b[:, c * 512:(c + 1) * 512],
                                            in0=ps, scalar1=rec, scalar2=None,
                                            op0=ALU.mult)
                wv_sb.append(wv_b)

            # wvT tiles [128, 16] bf16 (cols b*8+h)
            wvT = []
            for it in range(NI):
                tp = ps_tp.tile([128, 2 * H], BF16, tag="tp", name="tp_wv")
                for b in range(BP):
                    nc.tensor.transpose(out=tp[:, b * H:(b + 1) * H],
                                        in_=wv_sb[b][:, it * 128:(it + 1) * 128],
                                        identity=id_b[0:H, 0:H])
                t = mpool.tile([128, 2 * H], BF16, tag="wvT", bufs=10, name="wvT_t")
                nc.vector.tensor_copy(out=t, in_=tp)
                wvT.append(t)

            # vals full-form: [16, 1024] = wvals @ Wv; diag blocks via h::8 slices
            wv_w = load_w(f"wv{l}", NI)
            bv_row = load_brow(f"wv{l}")
            vch = [ps_wide.tile([2 * H, 512], F32, tag="wide", name=f"vch{c}")
                   for c in range(2)]
            for it in range(NI):
                w = wv_w(it)
                for c in range(2):
                    nc.tensor.matmul(vch[c], lhsT=wvT[it],
                                     rhs=w[:, c * 512:(c + 1) * 512],
                                     start=(it == 0),
                                     stop=(it == NI - 1 and bv_row is None))
            if bv_row is not None:
                for c in range(2):
                    nc.tensor.matmul(vch[c], lhsT=ones16,
                                     rhs=bv_row[:, c * 512:(c + 1) * 512],
                                     start=False, stop=True)
            vf = mpool.tile([2 * H, NZ], BF16, tag="vf", bufs=1, name="vf")
            for c in range(2):
                nc.scalar.copy(out=vf[:, c * 512:(c + 1) * 512], in_=vch[c])
            valsT = []
            for it in range(NI):
                tp = ps_tp.tile([128, 2 * H], BF16, tag="tp", name="tp_vf")
                nc.tensor.transpose(out=tp, in_=vf[:, it * 128:(it + 1) * 128],
                                    identity=id_b[0:2 * H, 0:2 * H])
                t = mpool.tile([128, 2 * H], BF16, tag="vT", bufs=10, name="vT_t")
                nc.vector.tensor_copy(out=t, in_=tp)
                valsT.append(t)
            vals_cols = [valsT[h][:, h::H] for h in range(H)]

            # attn_out (row form)
            raw_bf, raw_32 = linear_row(vals_cols, f"wao{l}", NI,
                                        want_bf=last, want32=not last)

            if not last:
                # x = LN(raw); pred; query = LN(pred(x) + query)
                x_bf, _ = ln_row(raw_32)
                p_cols = to_cols(x_bf)
                for i in range(3):
                    x_bf, _ = linear_row(p_cols, f"pred{l}_{i}", NI)
                    p_cols = to_cols(x_bf, act="lrelu")
                _, p32 = linear_row(p_cols, f"pred{l}_3", NI,
                                    want_bf=False, want32=True)
                qnew = rpool.tile([BP, NZ], F32, tag="r32", bufs=4, name="qnew")
                nc.vector.tensor_add(qnew, p32, query_32)
                query_bf, query_32 = ln_row(qnew, act_bf=True, want32=True)

        # ---- out = raw @ Wout : row form is already the output layout --
        out_cols = to_cols(raw_bf)
        _, out32 = linear_row(out_cols, "wout", NI, want_bf=False, want32=True)
        nc.sync.dma_start(out=out_d[:, :], in_=out32)

    nc.compile()
    return nc


def kernel(values, keys, query_input, start_ind, end_ind, params, _trace=False):
    in_maps, cfg = _host_prep(values, keys, query_input, start_ind, end_ind, params)
    nc = build(cfg)
    res = run_bass_kernel_spmd(nc, in_maps, core_ids=list(range(NCORES)), trace=_trace)
    LAST_RESULT["exec_time_ns"] = res.exec_time_ns
    LAST_RESULT["mean_exec_time_ns"] = res.mean_exec_time_ns
    LAST_RESULT["profile_json"] = res.profile_json
    out = np.concatenate([res.results[c]["out"] for c in range(NCORES)], axis=0)
    attw = np.concatenate([res.results[c]["attw"] for c in range(NCORES)], axis=0)
    return out.astype(np.float32), attw.astype(np.float32)


# revision 31
# speedup vs baseline: 1.1447x; 1.1447x over previous
"""Trainium2 Bass kernel for the 2-layer cross-attention module.

Sharding: data-parallel over batch B=16 -> 2 batch elements per core, 8 cores,
no collectives. Per core the algebra is restructured by linearity so the two
giant [T,NZ]@[NZ,NZ] projections disappear:

  scores[t,h] = sum_i keys[t,i] * qk[i,h],   qk[i,h] = sum_{d in head h} Wk[i,hd] q[hd]
  vals[hd]    = sum_i wvals[h,i] * Wv[i,hd], wvals[h,i] = sum_t p[t,h] values[t,i]

The small linear chain runs in "row form": activations [2, 1024] with the
column-layout x tiles as the (cheap) stationary operand and the weight matrix
moving in N=512 chunks — 8 LDWEIGHTS + 16 dense matmuls per linear instead of
64+64. The final layer's pred-MLP/LayerNorms are dead code in the reference
(query is discarded) and are skipped. All matmul inputs bf16 (f32 PSUM);
softmax/layernorm in f32.
"""

import numpy as np
import ml_dtypes

import concourse.mybir as mybir
from concourse import bacc, tile
from concourse.bass_utils import run_bass_kernel_spmd
from concourse.masks import make_identity

F32 = mybir.dt.float32
BF16 = mybir.dt.bfloat16
ALU = mybir.AluOpType
AXX = mybir.AxisListType
AF = mybir.ActivationFunctionType

B, T, NZ, H, DK = 16, 2048, 1024, 8, 128
NCORES, BP = 8, 2
NT, NI = T // 128, NZ // 128  # 16, 8
SCALE = float(1.0 / np.sqrt(DK))  # * attention_temperature (1.0)
EPS = 1e-5
N_LAYERS = 2
LRELU = 0.2

LAST_RESULT = {}  # test harness introspection: exec_time_ns etc.


def _np(x):
    return np.asarray(x)


def _bf(x):
    return np.ascontiguousarray(np.asarray(x, dtype=np.float32).astype(ml_dtypes.bfloat16))


def _f32(x):
    return np.ascontiguousarray(np.asarray(x), dtype=np.float32)


def _host_prep(values, keys, query_input, start_ind, end_ind, params):
    """Marshal full inputs into per-core shard dicts + build config."""
    values = _np(values)
    keys = _np(keys)
    query_input = _np(query_input)
    start_ind = _f32(start_ind)
    end_ind = _f32(end_ind)

    lin_w = {}
    lin_b = {}

    def add_lin(name, p):
        W, b = p
        lin_w[name] = _bf(_f32(W))
        lin_b[name] = _f32(b).reshape(1, -1)

    for i, p in enumerate(params["query_net"]):
        add_lin(f"qn{i}", p)
    for l, lp in enumerate(params["layers"]):
        add_lin(f"wq{l}", lp["q"])
        add_lin(f"wv{l}", lp["v"])
        add_lin(f"wao{l}", lp["attn_out"])
        if l != N_LAYERS - 1:  # last layer's pred MLP is dead code
            for i, p in enumerate(lp["pred"]):
                add_lin(f"pred{l}_{i}", p)
        Wk, bk = lp["k"]
        lin_w[f"wkT{l}"] = _bf(_f32(Wk).T)
        lin_b[f"bk{l}"] = _f32(bk).reshape(1, -1)
    add_lin("wout", params["out"])

    bias_nonzero = {k: bool(np.any(v)) for k, v in lin_b.items()}

    tt = np.arange(T, dtype=np.float32)
    mask = (tt[None, :] < np.floor(start_ind)[:, None]) | (
        tt[None, :] > np.ceil(end_ind)[:, None]
    )
    fully = mask.all(axis=1)
    maskadd = np.where(mask, np.float32(-1e30), np.float32(0.0)).astype(np.float32)
    maskadd[fully] = 0.0  # fully-masked row -> uniform softmax (matches reference)
    mask_nontrivial = bool(maskadd.any())

    cfg = {"bias_nonzero": bias_nonzero, "mask": mask_nontrivial}

    in_maps = []
    for c in range(NCORES):
        sl_ = slice(BP * c, BP * (c + 1))
        m = {
            "keysT": _bf(keys[sl_].transpose(0, 2, 1)),          # [BP, NZ, T]
            "values": _bf(values[sl_]),                          # [BP, T, NZ]
            "qin": _bf(query_input[sl_].T.reshape(16, 128, BP).transpose(1, 0, 2)),
        }
        for k, v in lin_w.items():
            m[k] = v
        for k, v in lin_b.items():
            if bias_nonzero[k]:
                if k.startswith("bk"):
                    m[k + "_col"] = v.reshape(H, DK).T.copy()
                else:
                    m[k + "_bias"] = v
        if mask_nontrivial:
            m["maskadd"] = np.repeat(maskadd[sl_][:, None, :], H, axis=1).copy()
        in_maps.append(m)
    return in_maps, cfg


def build(cfg):
    bias_nz = cfg["bias_nonzero"]
    nc = bacc.Bacc("TRN2", target_bir_lowering=False, debug=False)

    keysT_d = nc.dram_tensor("keysT", [BP, NZ, T], BF16, kind="ExternalInput")
    values_d = nc.dram_tensor("values", [BP, T, NZ], BF16, kind="ExternalInput")
    qin_d = nc.dram_tensor("qin", [128, 16, BP], BF16, kind="ExternalInput")

    w_d = {}
    b_d = {}

    def decl_lin(name, din):
        w_d[name] = nc.dram_tensor(name, [din, NZ], BF16, kind="ExternalInput")
        if bias_nz.get(name, False):
            b_d[name] = nc.dram_tensor(name + "_bias", [1, NZ], F32, kind="ExternalInput")

    decl_lin("qn0", 2 * NZ)
    for i in range(1, 5):
        decl_lin(f"qn{i}", NZ)
    for l in range(N_LAYERS):
        decl_lin(f"wq{l}", NZ)
        decl_lin(f"wv{l}", NZ)
        decl_lin(f"wao{l}", NZ)
        if l != N_LAYERS - 1:
            for i in range(4):
                decl_lin(f"pred{l}_{i}", NZ)
        w_d[f"wkT{l}"] = nc.dram_tensor(f"wkT{l}", [NZ, NZ], BF16, kind="ExternalInput")
        if bias_nz.get(f"bk{l}", False):
            b_d[f"bk{l}"] = nc.dram_tensor(f"bk{l}_col", [128, H], F32, kind="ExternalInput")
    decl_lin("wout", NZ)
    if cfg["mask"]:
        mask_d = nc.dram_tensor("maskadd", [BP, H, T], F32, kind="ExternalInput")

    out_d = nc.dram_tensor("out", [BP, NZ], F32, kind="ExternalOutput")
    attw_d = nc.dram_tensor("attw", [BP, T], F32, kind="ExternalOutput")

    from contextlib import ExitStack

    with tile.TileContext(nc) as tc, ExitStack() as ctx:
        singles = ctx.enter_context(tc.tile_pool(name="singles", bufs=1))
        wpool = ctx.enter_context(tc.tile_pool(name="wpool", bufs=9))
        apool = ctx.enter_context(tc.tile_pool(name="apool", bufs=36))
        rpool = ctx.enter_context(tc.tile_pool(name="rpool", bufs=4))
        ppool = ctx.enter_context(tc.tile_pool(name="ppool", bufs=2))
        mpool = ctx.enter_context(tc.tile_pool(name="mpool", bufs=3))
        ps_wide = ctx.enter_context(tc.tile_pool(name="ps_wide", bufs=4, space="PSUM"))
        ps_mm2 = ctx.enter_context(tc.tile_pool(name="ps_mm2", bufs=1, space="PSUM"))
        ps_tp = ctx.enter_context(tc.tile_pool(name="ps_tp", bufs=3, space="PSUM"))

        # ---- constants -------------------------------------------------
        id_b = singles.tile([128, 128], BF16, tag="id_b")
        make_identity(nc, id_b)
        ones2 = singles.tile([1, 2], F32, tag="ones2")
        nc.vector.memset(ones2, 1.0)
        ones16 = singles.tile([1, 16], F32, tag="ones16")
        nc.vector.memset(ones16, 1.0)
        sel8 = singles.tile([8, 1], BF16, tag="sel8")
        nc.vector.memset(sel8, 1.0 / H)
        eps_t = singles.tile([BP, 1], F32, tag="eps")
        nc.vector.memset(eps_t, EPS)

        # ---- persistent inputs (DMA deferred; weights queue first) -----
        keysT_sb3 = [singles.tile([128, NI, T], BF16, tag=f"kT{b}", name=f"kT{b}")
                     for b in range(BP)]
        keysT_sb = [[keysT_sb3[b][:, it, :] for it in range(NI)] for b in range(BP)]

        def load_keysT(b):
            nc.sync.dma_start(out=keysT_sb3[b],
                              in_=keysT_d[b].rearrange("(it p) t -> p it t", p=128))

        vspool = ctx.enter_context(tc.tile_pool(name="vspool", bufs=3))

        def stream_values(b, c, half):
            """[128, 8, 512] bf16: tt in [8*half, 8*half+8), cols c*512.."""
            v = vspool.tile([128, 8, 512], BF16, tag="vs", name="vs")
            nc.sync.dma_start(
                out=v,
                in_=values_d[b, half * 1024:(half + 1) * 1024,
                             c * 512:(c + 1) * 512].rearrange(
                                 "(tt p) i -> p tt i", p=128))
            return v

        qin_sb = singles.tile([128, 16, BP], BF16, tag="qin")
        nc.sync.dma_start(out=qin_sb, in_=qin_d[:, :, :])
        if cfg["mask"]:
            mask_sb = [singles.tile([H, T], F32, tag=f"mask{b}", name=f"mask{b}")
                       for b in range(BP)]
            for b in range(BP):
                nc.sync.dma_start(out=mask_sb[b], in_=mask_d[b, :, :])

        # ---- helpers ---------------------------------------------------
        def load_w(name, nk):
            segs = []
            for s0 in range(0, nk, 4):
                w = wpool.tile([128, 4, NZ], BF16, tag="w", name=f"w_{name}_{s0}")
                nc.sync.dma_start(
                    out=w,
                    in_=w_d[name][s0 * 128:(s0 + 4) * 128, :].rearrange(
                        "(k p) n -> p k n", p=128))
                segs.append(w)
            return lambda kt: segs[kt // 4][:, kt % 4, :]

        def load_brow(name):
            if not bias_nz.get(name, False):
                return None
            t = mpool.tile([1, NZ], F32, tag="brow", bufs=3, name=f"b_{name}")
            nc.sync.dma_start(out=t, in_=b_d[name][:, :])
            return t

        def linear_row(x_cols, name, nk, act=None, want_bf=True, want32=False):
            """Row-form linear: x_cols (nk [128,2] bf16 APs) @ W -> rows [2, NZ].

            Returns (row_bf16 | None, row_f32 | None).
            """
            wts = load_w(name, nk)
            brow = load_brow(name)
            ch = [ps_wide.tile([BP, 512], F32, tag="wide", name=f"r{c}_{name}")
                  for c in range(2)]
            for kt in range(nk):
                w = wts(kt)
                for c in range(2):
                    nc.tensor.matmul(ch[c], lhsT=x_cols[kt],
                                     rhs=w[:, c * 512:(c + 1) * 512],
                                     start=(kt == 0),
                                     stop=(kt == nk - 1 and brow is None))
            if brow is not None:
                for c in range(2):
                    nc.tensor.matmul(ch[c], lhsT=ones2,
                                     rhs=brow[:, c * 512:(c + 1) * 512],
                                     start=False, stop=True)
            # evict: ACT psum->SBUF with cast; lrelu (if any) is applied later
            # on the transposed col tiles to keep this boundary short
            r32 = None
            if want32:
                r32 = rpool.tile([BP, NZ], F32, tag="r32", bufs=4, name=f"r32_{name}")
                for c in range(2):
                    nc.scalar.copy(out=r32[:, c * 512:(c + 1) * 512], in_=ch[c])
            rbf = None
            if want_bf:
                rbf = rpool.tile([BP, NZ], BF16, tag="rbf", bufs=3, name=f"rbf_{name}")
                if r32 is not None:
                    nc.scalar.copy(out=rbf, in_=r32)
                else:
                    for c in range(2):
                        nc.scalar.copy(out=rbf[:, c * 512:(c + 1) * 512], in_=ch[c])
            return rbf, r32

        def to_cols(row_bf, n=NI, act=None):
            """[2, n*128] bf16 row -> list of n [128, 2] bf16 col tiles."""
            cols = []
            for kt in range(n):
                tp = ps_tp.tile([128, BP], BF16, tag="tp", name="tp_c")
                nc.tensor.transpose(out=tp, in_=row_bf[:, kt * 128:(kt + 1) * 128],
                                    identity=id_b[0:BP, 0:BP])
                t = apool.tile([128, BP], BF16, tag="act", name="col")
                nc.vector.tensor_copy(out=t, in_=tp)
                if act == "lrelu":
                    nc.vector.scalar_tensor_tensor(out=t, in0=t, scalar=LRELU,
                                                   in1=t, op0=ALU.mult, op1=ALU.max)
                cols.append(t)
            return cols

        def ln_row(r32, act_bf=True, want32=False):
            """LayerNorm over the free dim of [2, NZ] f32 rows."""
            stats = mpool.tile([BP, 2, 6], F32, tag="lnst", name="lnst")
            for c in range(2):
                nc.vector.bn_stats(out=stats[:, c, :], in_=r32[:, c * 512:(c + 1) * 512])
            mv = mpool.tile([BP, 2], F32, tag="lnmv", name="lnmv")
            nc.vector.bn_aggr(out=mv, in_=stats)
            sd = mpool.tile([BP, 1], F32, tag="lnsd", name="lnsd")
            nc.scalar.activation(out=sd, in_=mv[:, 1:2], func=AF.Sqrt,
                                 bias=eps_t, scale=1.0)
            rinv = mpool.tile([BP, 1], F32, tag="lnri", name="lnri")
            nc.vector.reciprocal(rinv, sd)
            yb = y32 = None
            if act_bf:
                yb = rpool.tile([BP, NZ], BF16, tag="rbf", bufs=3, name="ln_rbf")
                nc.vector.tensor_scalar(out=yb, in0=r32, scalar1=mv[:, 0:1],
                                        scalar2=rinv, op0=ALU.subtract, op1=ALU.mult)
            if want32:
                y32 = rpool.tile([BP, NZ], F32, tag="r32", bufs=4, name="ln_r32")
                nc.vector.tensor_scalar(out=y32, in0=r32, scalar1=mv[:, 0:1],
                                        scalar2=rinv, op0=ALU.subtract, op1=ALU.mult)
            return yb, y32

        # ---- query_net -------------------------------------------------
        x_cols = [qin_sb[:, kt, :] for kt in range(16)]
        for i in range(4):
            rbf, _ = linear_row(x_cols, f"qn{i}", 16 if i == 0 else NI)
            x_cols = to_cols(rbf, act="lrelu")
        query_bf, query_32 = linear_row(x_cols, "qn4", NI, want_bf=True, want32=True)

        raw_bf = None
        pnorm = [None, None]

        for l in range(N_LAYERS):
            last = l == N_LAYERS - 1
            # q projection
            q_cols = to_cols(query_bf)
            q_rbf, _ = linear_row(q_cols, f"wq{l}", NI)
            q_cc = to_cols(q_rbf)

            # qk^T = blockdiag(q) @ WkT   [8, 1024] per batch
            wkT = load_w(f"wkT{l}", NI)
            if l == 0:
                load_keysT(0)
                load_keysT(1)
            qk_col = [[None] * NI for _ in range(BP)]
            for b in range(BP):
                qblk = []
                for kt in range(H):
                    qb = apool.tile([128, H], BF16, tag="qblk", bufs=20, name="qb")
                    nc.vector.memset(qb, 0.0)
                    nc.vector.tensor_copy(out=qb[:, kt:kt + 1],
                                          in_=q_cc[kt][:, b:b + 1])
                    qblk.append(qb)
                qkT_sb = mpool.tile([H, NZ], BF16, tag="qkT", bufs=2, name="qkT")
                for c in range(2):
                    ps = ps_wide.tile([H, 512], F32, tag="wide", name="ps_qkT")
                    for kt in range(H):
                        nc.tensor.matmul(ps, lhsT=qblk[kt],
                                         rhs=wkT(kt)[:, c * 512:(c + 1) * 512],
                                         start=(kt == 0), stop=(kt == H - 1))
                    nc.scalar.copy(out=qkT_sb[:, c * 512:(c + 1) * 512], in_=ps)
                for it in range(NI):
                    tp = ps_tp.tile([128, H], BF16, tag="tp", name="tp_qk")
                    nc.tensor.transpose(out=tp, in_=qkT_sb[:, it * 128:(it + 1) * 128],
                                        identity=id_b[0:H, 0:H])
                    qc = apool.tile([128, H], BF16, tag="qkcol", bufs=20, name="qkc")
                    nc.vector.tensor_copy(out=qc, in_=tp)
                    qk_col[b][it] = qc

            # scb: per-head constant from k-bias (zero in practice -> skipped)
            scb_sb = [None, None]
            if bias_nz.get(f"bk{l}", False):
                bkcol = mpool.tile([128, H], F32, tag="bkcol", name="bkcol")
                nc.sync.dma_start(out=bkcol, in_=b_d[f"bk{l}"][:, :])
                ones512 = singles.tile([1, 512], F32, tag="ones512")
                nc.vector.memset(ones512, 1.0)
                bk_bf = mpool.tile([128, H], BF16, tag="bkbf", name="bkbf")
                nc.vector.tensor_copy(out=bk_bf, in_=bkcol)
                for b in range(BP):
                    ps = ps_mm2.tile([1, H], F32, tag="mm2", name="ps_scb")
                    for h in range(H):
                        nc.tensor.matmul(ps[:, h:h + 1],
                                         lhsT=q_cc[h][:, b:b + 1],
                                         rhs=bk_bf[:, h:h + 1],
                                         start=(h == 0), stop=(h == H - 1))
                    s_ = mpool.tile([1, H], F32, tag="scb", name="scb")
                    nc.vector.tensor_copy(out=s_, in_=ps)
                    scb_sb[b] = s_

            # scores -> softmax -> pT -> wvals, pipelined per batch.
            # Softmax without max-subtraction: scores are O(1) here (and a
            # -1e30 mask exps to exactly 0), so exp cannot overflow; exp is
            # fused into the per-chunk PSUM eviction with accumulated partial
            # sums, and the 1/sum normalization is folded into the wvals
            # eviction and the attw selector.
            wv_sb = []
            for b in range(BP):
                probs = ppool.tile([H, T], BF16, tag="probs", name="probs")
                psums = mpool.tile([H, 4], F32, tag="psums", name="psums")
                for c in range(4):
                    ps = ps_wide.tile([H, 512], F32, tag="wide", name="ps_sc")
                    for it in range(NI):
                        nc.tensor.matmul(ps,
                                         lhsT=qk_col[b][it],
                                         rhs=keysT_sb[b][it][:, c * 512:(c + 1) * 512],
                                         start=(it == 0),
                                         stop=(it == NI - 1 and scb_sb[b] is None))
                    if scb_sb[b] is not None:
                        nc.tensor.matmul(ps, lhsT=scb_sb[b], rhs=ones512,
                                         start=False, stop=True)
                    if cfg["mask"]:
                        nc.vector.tensor_add(ps, ps,
                                             mask_sb[b][:, c * 512:(c + 1) * 512])
                    nc.scalar.activation(out=probs[:, c * 512:(c + 1) * 512], in_=ps,
                                         func=AF.Exp, bias=0.0, scale=SCALE,
                                         accum_out=psums[:, c:c + 1])
                ssum = mpool.tile([H, 1], F32, tag="ssum", name="ssum")
                nc.vector.reduce_sum(ssum, psums, axis=AXX.X)
                rec = mpool.tile([H, 1], F32, tag="rec", bufs=4, name="rec")
                nc.vector.reciprocal(rec, ssum)
                pnorm[b] = probs

                if last:
                    selr = mpool.tile([H, 1], BF16, tag="selr", name="selr")
                    nc.vector.tensor_scalar_mul(selr, rec, 1.0 / H)
                    for c in range(4):
                        ps = ps_wide.tile([1, 512], F32, tag="wide", name="ps_aw")
                        nc.tensor.matmul(ps, lhsT=selr,
                                         rhs=pnorm[b][:, c * 512:(c + 1) * 512],
                                         start=True, stop=True)
                        awc = mpool.tile([1, 512], F32, tag="awc", bufs=1, name="awc")
                        nc.vector.tensor_copy(out=awc, in_=ps)
                        nc.sync.dma_start(out=attw_d[b:b + 1, c * 512:(c + 1) * 512],
                                          in_=awc)

                pT_b = []
                for tt in range(NT):
                    tp = ps_tp.tile([128, H], BF16, tag="tp", name="tp_p")
                    nc.tensor.transpose(out=tp,
                                        in_=pnorm[b][:, tt * 128:(tt + 1) * 128],
                                        identity=id_b[0:H, 0:H])
                    t = mpool.tile([128, H], BF16, tag="pT", bufs=36, name="pT_t")
                    nc.vector.tensor_copy(out=t, in_=tp)
                    pT_b.append(t)
                wv_b = mpool.tile([H, NZ], BF16, tag="wv", bufs=2, name="wv_b")
                for c in range(2):
                    ps = ps_wide.tile([H, 512], F32, tag="wide", name="ps_wv")
                    for half in range(2):
                        vt = stream_values(b, c, half)
                        for t8 in range(8):
                            tt = half * 8 + t8
                            nc.tensor.matmul(ps,
                                             lhsT=pT_b[tt],
                                             rhs=vt[:, t8, :],
                                             start=(tt == 0), stop=(tt == NT - 1))
                    nc.vector.tensor_scalar(out=wv_b[:, c * 512:(c + 1) * 512],
                                            in0=ps, scalar1=rec, scalar2=None,
                                            op0=ALU.mult)
                wv_sb.append(wv_b)

            # wvT tiles [128, 16] bf16 (cols b*8+h)
            wvT = []
            for it in range(NI):
                tp = ps_tp.tile([128, 2 * H], BF16, tag="tp", name="tp_wv")
                for b in range(BP):
                    nc.tensor.transpose(out=tp[:, b * H:(b + 1) * H],
                                        in_=wv_sb[b][:, it * 128:(it + 1) * 128],
                                        identity=id_b[0:H, 0:H])
                t = mpool.tile([128, 2 * H], BF16, tag="wvT", bufs=10, name="wvT_t")
                nc.vector.tensor_copy(out=t, in_=tp)
                wvT.append(t)

            # vals full-form: [16, 1024] = wvals @ Wv; diag blocks via h::8 slices
            wv_w = load_w(f"wv{l}", NI)
            bv_row = load_brow(f"wv{l}")
            vch = [ps_wide.tile([2 * H, 512], F32, tag="wide", name=f"vch{c}")
                   for c in range(2)]
            for it in range(NI):
                w = wv_w(it)
                for c in range(2):
                    nc.tensor.matmul(vch[c], lhsT=wvT[it],
                                     rhs=w[:, c * 512:(c + 1) * 512],
                                     start=(it == 0),
                                     stop=(it == NI - 1 and bv_row is None))
            if bv_row is not None:
                for c in range(2):
                    nc.tensor.matmul(vch[c], lhsT=ones16,
                                     rhs=bv_row[:, c * 512:(c + 1) * 512],
                                     start=False, stop=True)
            vf = mpool.tile([2 * H, NZ], BF16, tag="vf", bufs=1, name="vf")
            for c in range(2):
                nc.scalar.copy(out=vf[:, c * 512:(c + 1) * 512], in_=vch[c])
            valsT = []
            for it in range(NI):
                tp = ps_tp.tile([128, 2 * H], BF16, tag="tp", name="tp_vf")
                nc.tensor.transpose(out=tp, in_=vf[:, it * 128:(it + 1) * 128],
                                    identity=id_b[0:2 * H, 0:2 * H])
                t = mpool.tile([128, 2 * H], BF16, tag="vT", bufs=10, name="vT_t")
                nc.vector.tensor_copy(out=t, in_=tp)
                valsT.append(t)
            vals_cols = [valsT[h][:, h::H] for h in range(H)]

            # attn_out (row form)
            raw_bf, raw_32 = linear_row(vals_cols, f"wao{l}", NI,
                                        want_bf=last, want32=not last)

            if not last:
                # x = LN(raw); pred; query = LN(pred(x) + query)
                x_bf, _ = ln_row(raw_32)
                p_cols = to_cols(x_bf)
                for i in range(3):
                    x_bf, _ = linear_row(p_cols, f"pred{l}_{i}", NI)
                    p_cols = to_cols(x_bf, act="lrelu")
                _, p32 = linear_row(p_cols, f"pred{l}_3", NI,
                                    want_bf=False, want32=True)
                qnew = rpool.tile([BP, NZ], F32, tag="r32", bufs=4, name="qnew")
                nc.vector.tensor_add(qnew, p32, query_32)
                query_bf, query_32 = ln_row(qnew, act_bf=True, want32=True)

        # ---- out = raw @ Wout : row form is already the output layout --
        out_cols = to_cols(raw_bf)
        _, out32 = linear_row(out_cols, "wout", NI, want_bf=False, want32=True)
        nc.sync.dma_start(out=out_d[:, :], in_=out32)

    nc.compile()
    return nc


def kernel(values, keys, query_input, start_ind, end_ind, params, _trace=False):
    in_maps, cfg = _host_prep(values, keys, query_input, start_ind, end_ind, params)
    nc = build(cfg)
    res = run_bass_kernel_spmd(nc, in_maps, core_ids=list(range(NCORES)), trace=_trace)
    LAST_RESULT["exec_time_ns"] = res.exec_time_ns
    LAST_RESULT["mean_exec_time_ns"] = res.mean_exec_time_ns
    LAST_RESULT["profile_json"] = res.profile_json
    out = np.concatenate([res.results[c]["out"] for c in range(NCORES)], axis=0)
    attw = np.concatenate([res.results[c]["attw"] for c in range(NCORES)], axis=0)
    return out.astype(np.float32), attw.astype(np.float32)
